# revision 32
# baseline (speedup 1.0000x reference)
"""Mamba2 (BareMambaLayer) forward on 8 TRN2 NeuronCores via Bass/Tile.

Shapes (hardcoded): embed_data [4, 4096, 1024], W_in [4384, 1024],
conv_w [2304, 4], conv_b [2304], dt_bias/A_log/D [32], norm_w [2048],
W_out [1024, 2048] -> out [4, 4096, 1024] f32.

Sharding: core c -> (batch bi = c//2, head-half hh = c%2; 16 heads each).
Each core computes its in_proj slice (row-layout GEMM for z|dt, col-layout
GEMM for x|B|C), depthwise conv+silu (col layout, per-partition weights),
the chunked SSD scan (chunk Q=128), gating, and a partial out-projection
(W_out columns for its heads, pre-scaled by norm_w). The per-row RMS-norm
factor commutes with the column-split matmul, so the host combines:
  out[bi] = rsqrt((ssq0+ssq1)/2048 + 1e-5) * (P0 + P1).

Matmul inputs are bf16 (fp32 PSUM accumulation); decay/state math is fp32.
"""

import os
import numpy as np

B, L, DM = 4, 4096, 1024
NH, HD, DS = 16, 64, 128      # heads per core, headdim, d_state
Q = 128                       # chunk length
QPS = 4                       # chunks per super-chunk
SCW = Q * QPS                 # super-chunk width (512)
NSC = L // SCW                # super-chunks (8)
D_CONV = 4

_CACHE = {}


# ---------------------------------------------------------------------------
# Bass program (single core; SPMD across 8 cores with per-core input data)
# ---------------------------------------------------------------------------

def _build_nc(l_total):
    import bass_rust
    import concourse.bass as bass
    import concourse.mybir as mybir
    import concourse.tile as tile
    from concourse.vector_clock import ScopedClock

    f32 = mybir.dt.float32
    bf = mybir.dt.bfloat16
    AT = mybir.ActivationFunctionType
    OP = mybir.AluOpType
    AX = mybir.AxisListType

    nsc = l_total // SCW
    nch = l_total // Q

    class SplitDrainTC(tile.TileContext):
        # walrus in this env accepts few sem-waits per instruction: hoist
        # excess waits onto single-wait nops emitted just before the inst.
        _WAIT_CAP = 1

        def _add_instruction(self, inst):
            si = inst.sync_info
            if (si and si.on_wait and len(si.on_wait) > self._WAIT_CAP
                    and inst.engine != mybir.EngineType.Unassigned):
                waits = list(si.on_wait)
                si.on_wait = waits[:self._WAIT_CAP]
                for w in waits[self._WAIT_CAP:]:
                    nop = mybir.InstNoOp(
                        name=f"waitnop-{self.nc.next_id()}",
                        engine=inst.engine,
                        ins=[], outs=[],
                        sync_info=bass_rust.SyncInfo(on_wait=[w], on_update=[]),
                    )
                    super()._add_instruction(nop)
            super()._add_instruction(inst)

        def _drain_and_barrier(self, tick_clock, wait_clock):
            drain_inst = self.nc.sync.drain()
            wait_clock.add_sem_waits(
                drain_inst.ins, ScopedClock({None: tick_clock.global_clock})
            )
            si = drain_inst.ins.sync_info
            waits = list(si.on_wait) if si and si.on_wait else []
            if len(waits) > 1:
                si.on_wait = [waits[0]]
                for w in waits[1:]:
                    nop = self.nc.sync.nop(nofuse=True)
                    nop.ins.sync_info = bass_rust.SyncInfo(on_wait=[w], on_update=[])
            self.nc.all_engine_barrier()
            popped = self.nc._tile_sem_poison_stack.pop()
            assert popped is self._sem_poison
            self.nc.clear_and_free_semaphores(list(self.sems.allocated().values()))
            self.nc.all_engine_barrier()

    nc = bass.Bass("TRN2")
    # --- inputs (per core) ---
    xt = nc.dram_tensor("xt", [DM, l_total], bf, kind="ExternalInput")
    wrow = nc.dram_tensor("wrow", [DM, 1040], bf, kind="ExternalInput")   # [z|dt]^T
    wcol = nc.dram_tensor("wcol", [DM, 1280], bf, kind="ExternalInput")   # [x|B|C]^T
    wout = nc.dram_tensor("wout", [1024, 1024], bf, kind="ExternalInput")  # (Wout*nw)^T
    convw = nc.dram_tensor("convw", [1280, D_CONV], f32, kind="ExternalInput")
    convb = nc.dram_tensor("convb", [1280, 1], f32, kind="ExternalInput")
    dtb = nc.dram_tensor("dtb", [128, NH], f32, kind="ExternalInput")     # bcast
    abc = nc.dram_tensor("abc", [128, NH], f32, kind="ExternalInput")     # -exp(A_log)
    dbc = nc.dram_tensor("dbc", [128, NH], f32, kind="ExternalInput")     # D
    tril = nc.dram_tensor("tril", [128, 128], f32, kind="ExternalInput")  # [s',t] s'<=t
    identb = nc.dram_tensor("identb", [128, 128], bf, kind="ExternalInput")
    identf = nc.dram_tensor("identf", [128, 128], f32, kind="ExternalInput")
    ones1 = nc.dram_tensor("ones1", [1, 128], f32, kind="ExternalInput")
    onesc = nc.dram_tensor("onesc", [128, 1], f32, kind="ExternalInput")
    # --- outputs ---
    out = nc.dram_tensor("out", [l_total, 1024], bf, kind="ExternalOutput")
    ssqo = nc.dram_tensor("ssq", [l_total, 1], f32, kind="ExternalOutput")

    with SplitDrainTC(nc) as tc, tc.tile_pool(name="wpool", bufs=1) as wpool, \
            tc.tile_pool(name="cpool", bufs=1) as cpool, \
            tc.tile_pool(name="state", bufs=1) as state, \
            tc.tile_pool(name="xtp", bufs=2) as xtp, \
            tc.tile_pool(name="xcvp", bufs=2) as xcvp, \
            tc.tile_pool(name="zp", bufs=2) as zp, \
            tc.tile_pool(name="dsp", bufs=2) as dsp, \
            tc.tile_pool(name="cvt", bufs=2) as cvt, \
            tc.tile_pool(name="ck1", bufs=1) as ck1, \
            tc.tile_pool(name="ck2", bufs=2) as ck2, \
            tc.tile_pool(name="gps", bufs=2, space="PSUM") as gps, \
            tc.tile_pool(name="sps", bufs=2, space="PSUM") as sps, \
            tc.tile_pool(name="yps", bufs=1, space="PSUM") as yps, \
            tc.tile_pool(name="drp", bufs=2, space="DRAM") as drp:

        # resident weights (k-tile major: [p, k, n])
        wrow_sb = wpool.tile([128, 8, 1040], bf)
        nc.sync.dma_start(out=wrow_sb, in_=wrow.rearrange("(k p) n -> p k n", p=128))
        wcol_sb = wpool.tile([128, 8, 1280], bf)
        nc.sync.dma_start(out=wcol_sb, in_=wcol.rearrange("(k p) n -> p k n", p=128))
        wout_sb = wpool.tile([128, 8, 1024], bf)
        nc.sync.dma_start(out=wout_sb, in_=wout.rearrange("(k p) n -> p k n", p=128))

        tril_sb = cpool.tile([128, 128], f32)
        nc.sync.dma_start(out=tril_sb, in_=tril[:, :])
        identb_sb = cpool.tile([128, 128], bf)
        nc.sync.dma_start(out=identb_sb, in_=identb[:, :])
        identf_sb = cpool.tile([128, 128], f32)
        nc.sync.dma_start(out=identf_sb, in_=identf[:, :])
        ones1_sb = cpool.tile([1, 128], f32)
        nc.sync.dma_start(out=ones1_sb, in_=ones1[:, :])
        onesc_sb = cpool.tile([128, 1], f32)
        nc.sync.dma_start(out=onesc_sb, in_=onesc[:, :])
        dtb_sb = cpool.tile([128, NH], f32)
        nc.sync.dma_start(out=dtb_sb, in_=dtb[:, :])
        abc_sb = cpool.tile([128, NH], f32)
        nc.sync.dma_start(out=abc_sb, in_=abc[:, :])
        dbc_sb = cpool.tile([128, NH], f32)
        nc.sync.dma_start(out=dbc_sb, in_=dbc[:, :])
        convw_sb = cpool.tile([128, 10, D_CONV], f32)
        nc.sync.dma_start(out=convw_sb, in_=convw.rearrange("(m p) k -> p m k", p=128))
        convb_sb = cpool.tile([128, 10, 1], f32)
        nc.sync.dma_start(out=convb_sb, in_=convb.rearrange("(m p) k -> p m k", p=128))

        H = state.tile([128, NH, HD], f32)          # SSM state, n on partitions
        nc.vector.memset(H, 0.0)
        carry = state.tile([128, 10, 3], bf)        # conv halo per col m-tile
        nc.vector.memset(carry, 0.0)

        xt_r = xt.rearrange("(k p) l -> p k l", p=128)

        for sc in range(nsc):
            t0 = sc * SCW
            xt_sb = xtp.tile([128, 8, SCW], bf)
            nc.sync.dma_start(out=xt_sb, in_=xt_r[:, :, t0:t0 + SCW])

            # ---- col GEMM (x|B|C)^T + conv + silu ----
            xcv_sb = xcvp.tile([128, 10, SCW], bf)
            for m in range(10):
                ps = gps.tile([128, SCW], f32, tag="gps")
                for k in range(8):
                    nc.tensor.matmul(
                        ps, lhsT=wcol_sb[:, k, m * 128:(m + 1) * 128],
                        rhs=xt_sb[:, k, :], start=(k == 0), stop=(k == 7))
                xpre = cvt.tile([128, 3 + SCW], bf, tag="xpre")
                nc.vector.tensor_copy(out=xpre[:, 0:3], in_=carry[:, m, :])
                nc.vector.tensor_copy(out=xpre[:, 3:3 + SCW], in_=ps)
                nc.vector.tensor_copy(out=carry[:, m, :], in_=xpre[:, SCW:SCW + 3])
                acc = cvt.tile([128, SCW], f32, tag="cacc")
                nc.vector.tensor_scalar(
                    out=acc, in0=xpre[:, 0:SCW],
                    scalar1=convw_sb[:, m, 0:1], scalar2=convb_sb[:, m, 0:1],
                    op0=OP.mult, op1=OP.add)
                for k in range(1, D_CONV):
                    nc.vector.scalar_tensor_tensor(
                        out=acc, in0=xpre[:, k:SCW + k],
                        scalar=convw_sb[:, m, k:k + 1], op0=OP.mult,
                        in1=acc, op1=OP.add)
                nc.scalar.activation(out=xcv_sb[:, m, :], in_=acc,
                                     func=AT.Sigmoid)
                nc.vector.tensor_tensor(out=xcv_sb[:, m, :],
                                        in0=xcv_sb[:, m, :], in1=acc,
                                        op=OP.mult)

            # ---- row GEMM (z|dt) ----
            z_sb = zp.tile([128, QPS, 1024], bf)
            dt_sb = dsp.tile([128, QPS, NH], f32, tag="dt")
            s_sb = dsp.tile([128, QPS, NH], f32, tag="s")
            for mt in range(QPS):
                for (n0, nw) in ((0, 512), (512, 512), (1024, NH)):
                    ps = gps.tile([128, nw], f32, tag="gps")
                    for k in range(8):
                        nc.tensor.matmul(
                            ps, lhsT=xt_sb[:, k, mt * 128:(mt + 1) * 128],
                            rhs=wrow_sb[:, k, n0:n0 + nw],
                            start=(k == 0), stop=(k == 7))
                    if n0 < 1024:
                        nc.scalar.activation(
                            out=z_sb[:, mt, n0:n0 + nw], in_=ps,
                            func=AT.Sigmoid)
                        nc.vector.tensor_tensor(
                            out=z_sb[:, mt, n0:n0 + nw],
                            in0=z_sb[:, mt, n0:n0 + nw], in1=ps, op=OP.mult)
                    else:
                        tdt = ck2.tile([128, NH], f32, tag="tdt")
                        nc.vector.tensor_tensor(
                            out=tdt, in0=ps, in1=dtb_sb, op=OP.add)
                        nc.scalar.activation(out=tdt, in_=tdt,
                                             func=AT.Sigmoid, scale=-1.0)
                        nc.scalar.activation(out=tdt, in_=tdt, func=AT.Ln)
                        nc.vector.tensor_scalar_mul(
                            dt_sb[:, mt, :], tdt, -1.0)
                        nc.vector.tensor_tensor(
                            out=s_sb[:, mt, :], in0=dt_sb[:, mt, :], in1=abc_sb,
                            op=OP.mult)

            # ---- scan over the 4 chunks ----
            for q in range(QPS):
                c = sc * QPS + q
                qsl = slice(q * Q, (q + 1) * Q)

                # ell (in-chunk inclusive cumsum of s), per-head decay scalars
                ell_ps = sps.tile([128, NH], f32, tag="sp")
                nc.tensor.matmul(ell_ps, lhsT=tril_sb, rhs=s_sb[:, q, :],
                                 start=True, stop=True)
                ell = ck2.tile([128, NH], f32, tag="ell")
                nc.vector.tensor_copy(out=ell, in_=ell_ps)
                lamc_ps = sps.tile([1, NH], f32, tag="sp")
                nc.tensor.matmul(lamc_ps, lhsT=onesc_sb, rhs=s_sb[:, q, :],
                                 start=True, stop=True)
                lam1 = ck2.tile([1, NH], f32, tag="lam1")
                nc.vector.tensor_copy(out=lam1, in_=lamc_ps)
                lam_ps = sps.tile([128, NH], f32, tag="sp")
                nc.tensor.matmul(lam_ps, lhsT=ones1_sb, rhs=lam1,
                                 start=True, stop=True)
                g_b = ck2.tile([128, NH], f32, tag="gb")
                nc.scalar.activation(out=g_b, in_=lam_ps, func=AT.Exp)
                ew = ck2.tile([128, NH], f32, tag="ew")
                nc.vector.tensor_tensor(out=ew, in0=lam_ps, in1=ell, op=OP.subtract)
                nc.scalar.activation(out=ew, in_=ew, func=AT.Exp)
                erow = ck2.tile([128, NH], f32, tag="erow")
                nc.scalar.activation(out=erow, in_=ell, func=AT.Exp)

                # G^T masked (shared across heads)
                g_ps = sps.tile([128, 128], f32, tag="sp")
                nc.tensor.matmul(g_ps, lhsT=xcv_sb[:, 8, qsl],
                                 rhs=xcv_sb[:, 9, qsl], start=True, stop=True)
                gm = ck2.tile([128, 128], f32, tag="gm")
                nc.vector.tensor_tensor(out=gm, in0=g_ps, in1=tril_sb, op=OP.mult)

                # B rows (for dH)
                br_ps = sps.tile([128, 128], bf, tag="sp")
                nc.tensor.transpose(br_ps, xcv_sb[:, 8, qsl], identb_sb)
                brow = ck2.tile([128, 128], bf, tag="brow")
                nc.vector.tensor_copy(out=brow, in_=br_ps)

                # x transposes -> U rows, x rows
                xrow = ck2.tile([128, NH, HD], bf, tag="xrow")
                u_sb = ck2.tile([128, NH, HD], bf, tag="u")
                for j in range(8):
                    xT_ps = sps.tile([128, 128], bf, tag="sp")
                    nc.tensor.transpose(xT_ps, xcv_sb[:, j, qsl], identb_sb)
                    nc.vector.tensor_copy(
                        out=xrow[:, 2 * j:2 * j + 2, :],
                        in_=xT_ps.rearrange("p (h d) -> p h d", h=2))
                    nc.vector.tensor_tensor(
                        out=u_sb[:, 2 * j:2 * j + 2, :],
                        in0=xT_ps.rearrange("p (h d) -> p h d", h=2),
                        in1=dt_sb[:, q, 2 * j:2 * j + 2, None].to_broadcast(
                            [128, 2, HD]),
                        op=OP.mult)

                # decay matrices Mt[s',t] = exp(min(ell_t - ell_s', 0)) * Gm
                ellT_ps = sps.tile([NH, 128], f32, tag="sp")
                nc.tensor.transpose(ellT_ps, ell, identf_sb)
                ellT = ck2.tile([NH, 128], f32, tag="ellT")
                nc.vector.tensor_copy(out=ellT, in_=ellT_ps)
                ellscr = drp.tile([NH, 128], f32, tag="ellscr")
                nc.sync.dma_start(out=ellscr, in_=ellT)
                f_sb = ck1.tile([128, NH, 128], f32)
                nc.sync.dma_start(
                    out=f_sb,
                    in_=ellscr[None, :, :].to_broadcast([128, NH, 128]))
                dmin = ck1.tile([128, NH, 128], f32)
                nc.vector.tensor_tensor(
                    out=dmin, in0=f_sb,
                    in1=ell[:, :, None].to_broadcast([128, NH, 128]),
                    op=OP.subtract)
                nc.vector.tensor_scalar(out=dmin, in0=dmin, scalar1=0.0,
                                        scalar2=None, op0=OP.min)
                nc.scalar.activation(out=dmin, in_=dmin, func=AT.Exp)
                mt_sb = ck1.tile([128, NH, 128], bf)
                nc.vector.tensor_tensor(
                    out=mt_sb, in0=dmin,
                    in1=gm[:, None, :].to_broadcast([128, NH, 128]), op=OP.mult)

                # H snapshot in bf16 for this chunk's cross term
                hb = ck2.tile([128, NH, HD], bf, tag="hb")
                nc.vector.tensor_copy(out=hb, in_=H)

                # per-head matmuls: y_local, cross P
                y_ps = yps.tile([128, NH, HD], f32, tag="y")
                for h in range(NH):
                    nc.tensor.matmul(y_ps[:, h, :], lhsT=mt_sb[:, h, :],
                                     rhs=u_sb[:, h, :], start=True, stop=True)
                p_ps = yps.tile([128, NH, HD], f32, tag="pdh")
                for h in range(NH):
                    nc.tensor.matmul(p_ps[:, h, :], lhsT=xcv_sb[:, 9, qsl],
                                     rhs=hb[:, h, :], start=True, stop=True)

                # y = D*x + y_local + e_row*P
                ysb = ck1.tile([128, NH, HD], f32, tag="ysb")
                nc.vector.tensor_tensor(
                    out=ysb, in0=xrow,
                    in1=dbc_sb[:, :, None].to_broadcast([128, NH, HD]), op=OP.mult)
                nc.vector.tensor_tensor(out=ysb, in0=ysb, in1=y_ps, op=OP.add)
                ycr = ck1.tile([128, NH, HD], f32, tag="ycr")
                nc.vector.tensor_tensor(
                    out=ycr, in0=p_ps,
                    in1=erow[:, :, None].to_broadcast([128, NH, HD]), op=OP.mult)
                nc.vector.tensor_tensor(out=ysb, in0=ysb, in1=ycr, op=OP.add)

                # state update: H = g*H + B_row^T @ (ew*U)
                uw = ck2.tile([128, NH, HD], bf, tag="uw")
                nc.vector.tensor_tensor(
                    out=uw, in0=u_sb,
                    in1=ew[:, :, None].to_broadcast([128, NH, HD]), op=OP.mult)
                dh_ps = yps.tile([128, NH, HD], f32, tag="pdh")
                for h in range(NH):
                    nc.tensor.matmul(dh_ps[:, h, :], lhsT=brow,
                                     rhs=uw[:, h, :], start=True, stop=True)
                nc.vector.tensor_tensor(
                    out=H, in0=H,
                    in1=g_b[:, :, None].to_broadcast([128, NH, HD]), op=OP.mult)
                nc.vector.tensor_tensor(out=H, in0=H, in1=dh_ps, op=OP.add)

                # gating + ssq + out projection (partial)
                yg = ck2.tile([128, 1024], bf, tag="yg")
                nc.vector.tensor_tensor(
                    out=yg, in0=ysb.rearrange("p h d -> p (h d)"),
                    in1=z_sb[:, q, :], op=OP.mult)
                sq = ck1.tile([128, 1024], f32, tag="sq")
                nc.vector.tensor_tensor(out=sq, in0=yg, in1=yg, op=OP.mult)
                ssqt = ck2.tile([128, 1], f32, tag="ssqt")
                nc.vector.tensor_reduce(out=ssqt, in_=sq, axis=AX.X, op=OP.add)
                nc.sync.dma_start(out=ssqo[c * Q:(c + 1) * Q, :], in_=ssqt)

                ygT = ck2.tile([128, 8, 128], bf, tag="ygT")
                for j in range(8):
                    yT_ps = sps.tile([128, 128], bf, tag="sp")
                    nc.tensor.transpose(
                        yT_ps, yg[:, j * 128:(j + 1) * 128], identb_sb)
                    nc.vector.tensor_copy(out=ygT[:, j, :], in_=yT_ps)

                osb = ck2.tile([128, 1024], bf, tag="osb")
                for o in range(2):
                    ops_ = gps.tile([128, 512], f32, tag="gps")
                    for k in range(8):
                        nc.tensor.matmul(
                            ops_, lhsT=ygT[:, k, :],
                            rhs=wout_sb[:, k, o * 512:(o + 1) * 512],
                            start=(k == 0), stop=(k == 7))
                    nc.vector.tensor_copy(out=osb[:, o * 512:(o + 1) * 512],
                                          in_=ops_)
                nc.sync.dma_start(out=out[c * Q:(c + 1) * Q, :], in_=osb)

    return nc


# ---------------------------------------------------------------------------
# host packing
# ---------------------------------------------------------------------------

def _pack_inputs(embed_data, W_in, conv_w, conv_b, dt_bias, A_log, D, norm_w,
                 W_out, l_total=L, nb=B):
    import ml_dtypes
    bfd = ml_dtypes.bfloat16
    A_full = (-np.exp(A_log.astype(np.float32))).astype(np.float32)

    tril = np.triu(np.ones((128, 128), np.float32))
    identf = np.eye(128, dtype=np.float32)
    identb = identf.astype(bfd)
    ones1 = np.ones((1, 128), np.float32)

    in_maps = []
    for c in range(2 * nb):
        bi, hh = c // 2, c % 2
        hsl = slice(hh * NH, (hh + 1) * NH)
        csl = slice(hh * 1024, (hh + 1) * 1024)
        xtv = np.ascontiguousarray(
            embed_data[bi, :l_total].T).astype(bfd)                   # [1024, L]
        wrow = np.ascontiguousarray(np.concatenate(
            [W_in[csl], W_in[4352 + hh * NH:4352 + (hh + 1) * NH]], 0).T
        ).astype(bfd)                                                  # [1024,1040]
        wcol = np.ascontiguousarray(np.concatenate(
            [W_in[2048 + hh * 1024:2048 + (hh + 1) * 1024],
             W_in[4096:4224], W_in[4224:4352]], 0).T).astype(bfd)      # [1024,1280]
        woutv = np.ascontiguousarray(
            (W_out[:, csl] * norm_w[csl][None, :]).T).astype(bfd)      # [1024,1024]
        cw = np.ascontiguousarray(np.concatenate(
            [conv_w[hh * 1024:(hh + 1) * 1024], conv_w[2048:2304]], 0)
        ).astype(np.float32)
        cb = np.ascontiguousarray(np.concatenate(
            [conv_b[hh * 1024:(hh + 1) * 1024], conv_b[2048:2304]], 0)
        ).astype(np.float32)[:, None]
        in_maps.append({
            "xt": xtv, "wrow": wrow, "wcol": wcol, "wout": woutv,
            "convw": cw, "convb": cb,
            "dtb": np.ascontiguousarray(
                np.broadcast_to(dt_bias[hsl], (128, NH))).astype(np.float32),
            "abc": np.ascontiguousarray(
                np.broadcast_to(A_full[hsl], (128, NH))).astype(np.float32),
            "dbc": np.ascontiguousarray(
                np.broadcast_to(D[hsl], (128, NH))).astype(np.float32),
            "tril": tril, "identb": identb, "identf": identf, "ones1": ones1,
            "onesc": np.ones((128, 1), np.float32),
        })
    return in_maps


# ---------------------------------------------------------------------------
# entry point
# ---------------------------------------------------------------------------

def _get_runner():
    """Build the sharded-jit runner once (nc trace + jit closure)."""
    if "runner" in _CACHE:
        return _CACHE["runner"]
    import time as _t
    import sys as _sys
    _t0 = _t.perf_counter()
    import jax
    import jax.numpy as jnp
    from jax.sharding import Mesh, NamedSharding, PartitionSpec
    from jax.experimental.shard_map import shard_map
    import concourse.bass2jax as b2j
    import concourse.mybir as mybir
    _t1 = _t.perf_counter()

    nc = _build_nc(L)
    _t2 = _t.perf_counter()

    b2j.install_neuronx_cc_hook()
    partition_name = (nc.partition_id_tensor.name
                      if nc.partition_id_tensor else None)
    in_names, out_names, out_avals, in_avals = [], [], [], []
    for alloc in nc.m.functions[0].allocations:
        if not isinstance(alloc, mybir.MemoryLocationSet):
            continue
        name = alloc.memorylocations[0].name
        if alloc.kind == "ExternalInput":
            if name != partition_name:
                in_names.append(name)
                in_avals.append(jax.core.ShapedArray(
                    tuple(alloc.tensor_shape), mybir.dt.np(alloc.dtype)))
        elif alloc.kind == "ExternalOutput":
            out_names.append(name)
            out_avals.append(jax.core.ShapedArray(
                tuple(alloc.tensor_shape), mybir.dt.np(alloc.dtype)))
    n_params, n_outs = len(in_names), len(out_avals)
    all_in = in_names + out_names + ([partition_name] if partition_name else [])

    def _body(*args):
        ops = list(args)
        if partition_name:
            ops.append(b2j.partition_id_tensor())
        outs = b2j._bass_exec_p.bind(
            *ops, out_avals=tuple(out_avals), in_names=tuple(all_in),
            out_names=tuple(out_names), lowering_input_output_aliases=(),
            sim_require_finite=True, sim_require_nnan=True, nc=nc)
        return tuple(outs)

    devices = jax.devices()[:8]
    mesh = Mesh(np.asarray(devices), ("core",))
    spec = NamedSharding(mesh, PartitionSpec("core"))
    in_specs = (PartitionSpec("core"),) * (n_params + n_outs)
    out_specs = (PartitionSpec("core"),) * n_outs
    donate = tuple(range(n_params, n_params + n_outs))
    sharded = jax.jit(
        shard_map(_body, mesh=mesh, in_specs=in_specs, out_specs=out_specs,
                  check_rep=False),
        donate_argnums=donate, keep_unused=True)
    zshapes = [(8 * a.shape[0], *a.shape[1:]) for a in out_avals]
    zdtypes = [a.dtype for a in out_avals]
    make_zeros = jax.jit(
        lambda: tuple(jnp.zeros(s, d) for s, d in zip(zshapes, zdtypes)),
        out_shardings=tuple(spec for _ in out_avals))

    runner = {
        "jax": jax, "sharded": sharded, "make_zeros": make_zeros,
        "spec": spec, "in_names": in_names, "out_names": out_names,
        "out_avals": out_avals, "in_avals": in_avals,
    }
    _CACHE["runner"] = runner
    return runner


def _get_prep_jits():
    """Device-side prep (dedup uploads via all-gather) and combine jits.
    Export-cached to disk so their compiled NEFFs are reused regardless of
    the directory kernel.py runs from (jax module hashes embed file paths)."""
    if "prep" in _CACHE:
        return _CACHE["prep"]
    import jax
    import jax.numpy as jnp
    from jax import export as jexport
    from jax.sharding import Mesh, NamedSharding, PartitionSpec as P
    from jax.experimental.shard_map import shard_map

    mesh = Mesh(np.asarray(jax.devices()[:8]), ("core",))
    spec = NamedSharding(mesh, P("core"))
    path = _export_cache_path() + "_pc"
    if os.path.exists(path + "_prep.bin") and os.path.exists(path + "_comb.bin"):
        prep_j = jax.jit(jexport.deserialize(
            open(path + "_prep.bin", "rb").read()).call)
        comb_j = jax.jit(jexport.deserialize(
            open(path + "_comb.bin", "rb").read()).call)
        _CACHE["prep"] = (prep_j, comb_j, spec)
        return _CACHE["prep"]

    PAIRS = [[0, 1], [2, 3], [4, 5], [6, 7]]
    QUADS = [[0, 2, 4, 6], [1, 3, 5, 7]]

    def prep(eh, wrq, wcq, woq):
        # eh [2048, 1024]: this core's L-half of its batch's bf16 embedding
        xt = jax.lax.all_gather(eh, "core", axis=0, axis_index_groups=PAIRS)
        xt = xt.reshape(4096, 1024).T
        # w*q [256, n]: this core's quarter of its head-half's weights
        wr = jax.lax.all_gather(wrq, "core", axis=0,
                                axis_index_groups=QUADS).reshape(1024, -1)
        wc = jax.lax.all_gather(wcq, "core", axis=0,
                                axis_index_groups=QUADS).reshape(1024, -1)
        wo = jax.lax.all_gather(woq, "core", axis=0,
                                axis_index_groups=QUADS).reshape(1024, -1)
        # donated zero output buffers for the bass call, made device-side
        zo = jnp.zeros((L, 1024), jnp.bfloat16)
        zs = jnp.zeros((L, 1), jnp.float32)
        return xt, wr, wc, wo, zo, zs

    prep_j = jax.jit(shard_map(
        prep, mesh=mesh, in_specs=(P("core"),) * 4, out_specs=(P("core"),) * 6))

    def comb(o, s):
        og = jax.lax.all_gather(o.astype(jnp.float32), "core",
                                axis_index_groups=PAIRS)
        sg = jax.lax.all_gather(s, "core", axis_index_groups=PAIRS)
        rr = jax.lax.rsqrt(sg.sum(0) / 2048.0 + 1e-5)
        res = (og.sum(0) * rr).astype(jnp.bfloat16)
        idx = jax.lax.axis_index("core") % 2
        return jax.lax.dynamic_slice_in_dim(res, idx * 2048, 2048, 0)

    comb_j = jax.jit(shard_map(
        comb, mesh=mesh, in_specs=(P("core"), P("core")), out_specs=P("core")))

    try:
        bfd = jnp.bfloat16
        prep_avals = (jax.ShapeDtypeStruct((8 * 2048, DM), bfd),
                      jax.ShapeDtypeStruct((8 * 256, 1040), bfd),
                      jax.ShapeDtypeStruct((8 * 256, 1280), bfd),
                      jax.ShapeDtypeStruct((8 * 256, 1024), bfd))
        comb_avals = (jax.ShapeDtypeStruct((8 * L, 1024), bfd),
                      jax.ShapeDtypeStruct((8 * L, 1), np.float32))
        with open(path + "_prep.bin", "wb") as f:
            f.write(jexport.export(prep_j)(*prep_avals).serialize())
        with open(path + "_comb.bin", "wb") as f:
            f.write(jexport.export(comb_j)(*comb_avals).serialize())
    except Exception:
        pass

    _CACHE["prep"] = (prep_j, comb_j, spec)
    return _CACHE["prep"]


def _export_cache_path():
    import hashlib
    import inspect
    src = inspect.getsource(_build_nc) + inspect.getsource(_get_prep_jits)
    h = hashlib.sha256(src.encode()).hexdigest()[:12]
    d = os.path.expanduser("~/.cache/mamba_trn")
    os.makedirs(d, exist_ok=True)
    return os.path.join(d, f"exp_{h}")


def _get_exec():
    """Jitted bass computation: from the export cache if present, else built
    fresh (and then cached). Returns (fn, in_names, out_names, out_gshapes)."""
    if "exec" in _CACHE:
        return _CACHE["exec"]
    import json
    import jax
    from jax import export as jexport
    import concourse.bass2jax as b2j
    b2j.BassEffect.__eq__ = lambda self, other: type(self) is type(other)
    b2j.BassEffect.__hash__ = lambda self: hash(type(self))
    b2j.install_neuronx_cc_hook()

    path = _export_cache_path()
    if os.path.exists(path + ".bin") and os.path.exists(path + ".json"):
        meta = json.load(open(path + ".json"))
        exp = jexport.deserialize(open(path + ".bin", "rb").read())
        fn = jax.jit(exp.call)
        res = (fn, meta["in_names"], meta["out_names"],
               [tuple(s) for s in meta["out_gshapes"]],
               [np.dtype(d) for d in meta["out_dtypes"]])
    else:
        r = _get_runner()
        avals = [jax.ShapeDtypeStruct((8 * a.shape[0], *a.shape[1:]), a.dtype)
                 for a in r["in_avals"]]
        zavals = [jax.ShapeDtypeStruct((8 * a.shape[0], *a.shape[1:]), a.dtype)
                  for a in r["out_avals"]]
        gshapes = [tuple(z.shape) for z in zavals]
        gdtypes = [str(np.dtype(z.dtype)) for z in zavals]
        try:
            exp = jexport.export(r["sharded"], disabled_checks=[
                jexport.DisabledSafetyCheck.custom_call("bass_exec")])(
                *avals, *zavals)
            with open(path + ".bin", "wb") as f:
                f.write(exp.serialize())
            with open(path + ".json", "w") as f:
                json.dump({"in_names": r["in_names"],
                           "out_names": r["out_names"],
                           "out_gshapes": gshapes,
                           "out_dtypes": gdtypes}, f)
        except Exception:
            pass
        res = (r["sharded"], r["in_names"], r["out_names"], gshapes,
               [np.dtype(z.dtype) for z in zavals])
    _CACHE["exec"] = res
    return res


def _run_on_trn(embed_data, W_in, conv_w, conv_b, dt_bias, A_log, D, norm_w,
                W_out):
    import time as _t
    import sys as _sys
    import ml_dtypes
    import jax
    bfd = ml_dtypes.bfloat16

    _t0 = _t.perf_counter()
    prep_j, comb_j, spec = _get_prep_jits()

    # ---- pack + async upload (embed conversion runs in a worker thread) ----
    from concurrent.futures import ThreadPoolExecutor
    pool = ThreadPoolExecutor(1)
    fut_eh = pool.submit(
        lambda: jax.device_put(
            embed_data.astype(bfd).reshape(8 * 2048, DM), spec))

    A_full = (-np.exp(A_log.astype(np.float32))).astype(np.float32)
    wrow_h, wcol_h, wout_h = [], [], []
    for hh in range(2):
        csl = slice(hh * 1024, (hh + 1) * 1024)
        wrow_h.append(np.concatenate(
            [W_in[csl], W_in[4352 + hh * NH:4352 + (hh + 1) * NH]], 0)
            .T.astype(bfd))                                # [1024, 1040]
        wcol_h.append(np.concatenate(
            [W_in[2048 + hh * 1024:2048 + (hh + 1) * 1024],
             W_in[4096:4224], W_in[4224:4352]], 0).T.astype(bfd))
        wout_h.append((W_out[:, csl] * norm_w[csl][None, :]).T.astype(bfd))

    def quarters(ws):
        return np.concatenate(
            [ws[c % 2][256 * (c // 2):256 * (c // 2 + 1)] for c in range(8)], 0)

    up = {
        "wrq": jax.device_put(quarters(wrow_h), spec),
        "wcq": jax.device_put(quarters(wcol_h), spec),
        "woq": jax.device_put(quarters(wout_h), spec),
    }

    tril = np.triu(np.ones((128, 128), np.float32))
    identf = np.eye(128, dtype=np.float32)
    smalls = {}
    for hh in range(2):
        hsl = slice(hh * NH, (hh + 1) * NH)
        smalls[hh] = {
            "convw": np.concatenate(
                [conv_w[hh * 1024:(hh + 1) * 1024], conv_w[2048:2304]],
                0).astype(np.float32),
            "convb": np.concatenate(
                [conv_b[hh * 1024:(hh + 1) * 1024], conv_b[2048:2304]],
                0).astype(np.float32)[:, None],
            "dtb": np.ascontiguousarray(np.broadcast_to(
                dt_bias[hsl], (128, NH))).astype(np.float32),
            "abc": np.ascontiguousarray(np.broadcast_to(
                A_full[hsl], (128, NH))).astype(np.float32),
            "dbc": np.ascontiguousarray(np.broadcast_to(
                D[hsl], (128, NH))).astype(np.float32),
        }
    shared = {"tril": tril, "identb": identf.astype(bfd), "identf": identf,
              "ones1": np.ones((1, 128), np.float32),
              "onesc": np.ones((128, 1), np.float32)}
    small_up = {}
    for n in ("convw", "convb", "dtb", "abc", "dbc"):
        small_up[n] = jax.device_put(
            np.concatenate([smalls[c % 2][n] for c in range(8)], 0), spec)
    for n, v in shared.items():
        small_up[n] = jax.device_put(
            np.concatenate([v] * 8, 0), spec)
    _t1 = _t.perf_counter()

    # ---- bass computation (export cache avoids re-tracing the program) ----
    fn, in_names, out_names, out_gshapes, out_gdtypes = _get_exec()
    _t2 = _t.perf_counter()

    xt_g, wr_g, wc_g, wo_g, zo, zs = prep_j(
        fut_eh.result(), up["wrq"], up["wcq"], up["woq"])
    big = {"xt": xt_g, "wrow": wr_g, "wcol": wc_g, "wout": wo_g}
    concat_in = [big[n] if n in big else small_up[n] for n in in_names]
    zeros = {"out": zo, "ssq": zs}
    outs = fn(*concat_in, *[zeros[n] for n in out_names])
    by_name = dict(zip(out_names, outs))
    final = comb_j(by_name["out"], by_name["ssq"])
    resv = np.asarray(final)

    return np.ascontiguousarray(
        resv.reshape(B, L, DM).astype(np.float32))


def _numpy_fallback(embed_data, W_in, conv_w, conv_b, dt_bias, A_log, D,
                    norm_w, W_out):
    """Vectorized chunked-SSD fallback (float32, CPU)."""
    b, l, _ = embed_data.shape
    d_inner, nheads, headdim = 2048, 32, 64
    d_state, nch = 128, l // Q
    zxbcdt = embed_data.reshape(b * l, DM) @ W_in.T
    zxbcdt = zxbcdt.reshape(b, l, -1)
    z = zxbcdt[..., :d_inner]
    xBC = zxbcdt[..., d_inner:d_inner + 2304]
    dtr = zxbcdt[..., d_inner + 2304:]
    xpad = np.concatenate([np.zeros((b, 3, 2304), np.float32), xBC], 1)
    xc = conv_b + sum(conv_w[:, k] * xpad[:, k:k + l] for k in range(D_CONV))
    xBC = xc / (1.0 + np.exp(-xc))
    xs = xBC[..., :d_inner].reshape(b, l, nheads, headdim)
    Bm = xBC[..., d_inner:d_inner + d_state]
    Cm = xBC[..., d_inner + d_state:]
    dt = np.where(dtr + dt_bias > 20.0, dtr + dt_bias,
                  np.log1p(np.exp(np.minimum(dtr + dt_bias, 20.0))))
    s = dt * (-np.exp(A_log))
    trilm = np.tril(np.ones((Q, Q), np.float32))
    y = np.empty((b, nch, Q, nheads, headdim), np.float32)
    for bi in range(b):
        ell = np.cumsum(s[bi].reshape(nch, Q, nheads), axis=1)
        lam = ell[:, -1, :]
        U = (dt[bi, :, :, None] * xs[bi]).reshape(nch, Q, nheads, headdim)
        B_c = Bm[bi].reshape(nch, Q, d_state)
        C_c = Cm[bi].reshape(nch, Q, d_state)
        G = np.einsum('ctn,csn->cts', C_c, B_c)
        Lm = np.exp(np.minimum(ell[:, :, None, :] - ell[:, None, :, :], 0.0))
        M = G[..., None] * Lm * trilm[None, :, :, None]
        yb = np.einsum('ctsh,cshp->cthp', M, U)
        dH = np.einsum('ctn,cth,cthp->chnp', B_c,
                       np.exp(lam[:, None, :] - ell), U)
        Hs = np.zeros((nheads, d_state, headdim), np.float32)
        expell = np.exp(ell)
        for c in range(nch):
            yb[c] += np.einsum('tn,hnp,th->thp', C_c[c], Hs, expell[c])
            Hs = np.exp(lam[c])[:, None, None] * Hs + dH[c]
        y[bi] = yb + D[None, None, :, None] * xs[bi].reshape(
            nch, Q, nheads, headdim)
    y = y.reshape(b, l, d_inner)
    y = y * (z / (1.0 + np.exp(-z)))
    ms = np.mean(np.square(y), axis=-1, keepdims=True)
    y = y / np.sqrt(ms + 1e-5) * norm_w
    return (y.reshape(b * l, d_inner) @ W_out.T).reshape(b, l, DM)


def kernel(embed_data, W_in, conv_w, conv_b, dt_bias, A_log, D, norm_w, W_out):
    args = [np.asarray(a, dtype=np.float32) for a in (
        embed_data, W_in, conv_w, conv_b, dt_bias, A_log, D, norm_w, W_out)]
    if os.environ.get("MAMBA_FORCE_NUMPY"):
        return _numpy_fallback(*args)
    try:
        return _run_on_trn(*args)
    except Exception:
        import traceback
        traceback.print_exc()
        return _numpy_fallback(*args)


# revision 34
# speedup vs baseline: 2.3100x; 2.3100x over previous
"""Mamba2 (BareMambaLayer) forward on 8 TRN2 NeuronCores via Bass/Tile.

Shapes (hardcoded): embed_data [4, 4096, 1024], W_in [4384, 1024],
conv_w [2304, 4], conv_b [2304], dt_bias/A_log/D [32], norm_w [2048],
W_out [1024, 2048] -> out [4, 4096, 1024] f32.

Sharding: core c -> (batch bi = c//2, head-half hh = c%2; 16 heads each).
Each core computes its in_proj slice (row-layout GEMM for z|dt, col-layout
GEMM for x|B|C), depthwise conv+silu (col layout, per-partition weights),
the chunked SSD scan (chunk Q=128), gating, and a partial out-projection
(W_out columns for its heads, pre-scaled by norm_w). The per-row RMS-norm
factor commutes with the column-split matmul, so the host combines:
  out[bi] = rsqrt((ssq0+ssq1)/2048 + 1e-5) * (P0 + P1).

Matmul inputs are bf16 (fp32 PSUM accumulation); decay/state math is fp32.
"""

import os
import numpy as np

B, L, DM = 4, 4096, 1024
NH, HD, DS = 16, 64, 128      # heads per core, headdim, d_state
Q = 128                       # chunk length
QPS = 4                       # chunks per super-chunk
SCW = Q * QPS                 # super-chunk width (512)
NSC = L // SCW                # super-chunks (8)
D_CONV = 4

_CACHE = {}


# ---------------------------------------------------------------------------
# Bass program (single core; SPMD across 8 cores with per-core input data)
# ---------------------------------------------------------------------------

def _build_nc(l_total):
    import bass_rust
    import concourse.bass as bass
    import concourse.mybir as mybir
    import concourse.tile as tile
    from concourse.vector_clock import ScopedClock

    f32 = mybir.dt.float32
    bf = mybir.dt.bfloat16
    AT = mybir.ActivationFunctionType
    OP = mybir.AluOpType
    AX = mybir.AxisListType

    nsc = l_total // SCW
    nch = l_total // Q

    class SplitDrainTC(tile.TileContext):
        # walrus in this env accepts few sem-waits per instruction: hoist
        # excess waits onto single-wait nops emitted just before the inst.
        _WAIT_CAP = 1

        def _add_instruction(self, inst):
            si = inst.sync_info
            if (si and si.on_wait and len(si.on_wait) > self._WAIT_CAP
                    and inst.engine != mybir.EngineType.Unassigned):
                waits = list(si.on_wait)
                si.on_wait = waits[:self._WAIT_CAP]
                for w in waits[self._WAIT_CAP:]:
                    nop = mybir.InstNoOp(
                        name=f"waitnop-{self.nc.next_id()}",
                        engine=inst.engine,
                        ins=[], outs=[],
                        sync_info=bass_rust.SyncInfo(on_wait=[w], on_update=[]),
                    )
                    super()._add_instruction(nop)
            super()._add_instruction(inst)

        def _drain_and_barrier(self, tick_clock, wait_clock):
            drain_inst = self.nc.sync.drain()
            wait_clock.add_sem_waits(
                drain_inst.ins, ScopedClock({None: tick_clock.global_clock})
            )
            si = drain_inst.ins.sync_info
            waits = list(si.on_wait) if si and si.on_wait else []
            if len(waits) > 1:
                si.on_wait = [waits[0]]
                for w in waits[1:]:
                    nop = self.nc.sync.nop(nofuse=True)
                    nop.ins.sync_info = bass_rust.SyncInfo(on_wait=[w], on_update=[])
            self.nc.all_engine_barrier()
            popped = self.nc._tile_sem_poison_stack.pop()
            assert popped is self._sem_poison
            self.nc.clear_and_free_semaphores(list(self.sems.allocated().values()))
            self.nc.all_engine_barrier()

    nc = bass.Bass("TRN2")
    # --- inputs (per core) ---
    xt = nc.dram_tensor("xt", [DM, l_total], bf, kind="ExternalInput")
    wrow = nc.dram_tensor("wrow", [DM, 1040], bf, kind="ExternalInput")   # [z|dt]^T
    wcol = nc.dram_tensor("wcol", [DM, 1280], bf, kind="ExternalInput")   # [x|B|C]^T
    wout = nc.dram_tensor("wout", [1024, 1024], bf, kind="ExternalInput")  # (Wout*nw)^T
    convw = nc.dram_tensor("convw", [1280, D_CONV], f32, kind="ExternalInput")
    convb = nc.dram_tensor("convb", [1280, 1], f32, kind="ExternalInput")
    dtb = nc.dram_tensor("dtb", [128, NH], f32, kind="ExternalInput")     # bcast
    abc = nc.dram_tensor("abc", [128, NH], f32, kind="ExternalInput")     # -exp(A_log)
    dbc = nc.dram_tensor("dbc", [128, NH], f32, kind="ExternalInput")     # D
    tril = nc.dram_tensor("tril", [128, 128], f32, kind="ExternalInput")  # [s',t] s'<=t
    identb = nc.dram_tensor("identb", [128, 128], bf, kind="ExternalInput")
    identf = nc.dram_tensor("identf", [128, 128], f32, kind="ExternalInput")
    ones1 = nc.dram_tensor("ones1", [1, 128], f32, kind="ExternalInput")
    onesc = nc.dram_tensor("onesc", [128, 1], f32, kind="ExternalInput")
    # --- outputs ---
    out = nc.dram_tensor("out", [l_total, 1024], bf, kind="ExternalOutput")
    ssqo = nc.dram_tensor("ssq", [l_total, 1], f32, kind="ExternalOutput")

    with SplitDrainTC(nc) as tc, tc.tile_pool(name="wpool", bufs=1) as wpool, \
            tc.tile_pool(name="cpool", bufs=1) as cpool, \
            tc.tile_pool(name="state", bufs=1) as state, \
            tc.tile_pool(name="xtp", bufs=2) as xtp, \
            tc.tile_pool(name="xcvp", bufs=2) as xcvp, \
            tc.tile_pool(name="zp", bufs=2) as zp, \
            tc.tile_pool(name="dsp", bufs=2) as dsp, \
            tc.tile_pool(name="cvt", bufs=2) as cvt, \
            tc.tile_pool(name="ck1", bufs=1) as ck1, \
            tc.tile_pool(name="ck2", bufs=2) as ck2, \
            tc.tile_pool(name="gps", bufs=2, space="PSUM") as gps, \
            tc.tile_pool(name="sps", bufs=2, space="PSUM") as sps, \
            tc.tile_pool(name="yps", bufs=1, space="PSUM") as yps, \
            tc.tile_pool(name="drp", bufs=2, space="DRAM") as drp:

        # resident weights (k-tile major: [p, k, n])
        wrow_sb = wpool.tile([128, 8, 1040], bf)
        nc.sync.dma_start(out=wrow_sb, in_=wrow.rearrange("(k p) n -> p k n", p=128))
        wcol_sb = wpool.tile([128, 8, 1280], bf)
        nc.sync.dma_start(out=wcol_sb, in_=wcol.rearrange("(k p) n -> p k n", p=128))
        wout_sb = wpool.tile([128, 8, 1024], bf)
        nc.sync.dma_start(out=wout_sb, in_=wout.rearrange("(k p) n -> p k n", p=128))

        tril_sb = cpool.tile([128, 128], f32)
        nc.sync.dma_start(out=tril_sb, in_=tril[:, :])
        identb_sb = cpool.tile([128, 128], bf)
        nc.sync.dma_start(out=identb_sb, in_=identb[:, :])
        identf_sb = cpool.tile([128, 128], f32)
        nc.sync.dma_start(out=identf_sb, in_=identf[:, :])
        ones1_sb = cpool.tile([1, 128], f32)
        nc.sync.dma_start(out=ones1_sb, in_=ones1[:, :])
        onesc_sb = cpool.tile([128, 1], f32)
        nc.sync.dma_start(out=onesc_sb, in_=onesc[:, :])
        dtb_sb = cpool.tile([128, NH], f32)
        nc.sync.dma_start(out=dtb_sb, in_=dtb[:, :])
        abc_sb = cpool.tile([128, NH], f32)
        nc.sync.dma_start(out=abc_sb, in_=abc[:, :])
        dbc_sb = cpool.tile([128, NH], f32)
        nc.sync.dma_start(out=dbc_sb, in_=dbc[:, :])
        convw_sb = cpool.tile([128, 10, D_CONV], f32)
        nc.sync.dma_start(out=convw_sb, in_=convw.rearrange("(m p) k -> p m k", p=128))
        convb_sb = cpool.tile([128, 10, 1], f32)
        nc.sync.dma_start(out=convb_sb, in_=convb.rearrange("(m p) k -> p m k", p=128))

        H = state.tile([128, NH, HD], f32)          # SSM state, n on partitions
        nc.vector.memset(H, 0.0)
        carry = state.tile([128, 10, 3], bf)        # conv halo per col m-tile
        nc.vector.memset(carry, 0.0)

        xt_r = xt.rearrange("(k p) l -> p k l", p=128)

        for sc in range(nsc):
            t0 = sc * SCW
            xt_sb = xtp.tile([128, 8, SCW], bf)
            nc.sync.dma_start(out=xt_sb, in_=xt_r[:, :, t0:t0 + SCW])

            # ---- col GEMM (x|B|C)^T + conv + silu ----
            xcv_sb = xcvp.tile([128, 10, SCW], bf)
            for m in range(10):
                ps = gps.tile([128, SCW], f32, tag="gps")
                for k in range(8):
                    nc.tensor.matmul(
                        ps, lhsT=wcol_sb[:, k, m * 128:(m + 1) * 128],
                        rhs=xt_sb[:, k, :], start=(k == 0), stop=(k == 7))
                xpre = cvt.tile([128, 3 + SCW], bf, tag="xpre")
                nc.vector.tensor_copy(out=xpre[:, 0:3], in_=carry[:, m, :])
                nc.vector.tensor_copy(out=xpre[:, 3:3 + SCW], in_=ps)
                nc.vector.tensor_copy(out=carry[:, m, :], in_=xpre[:, SCW:SCW + 3])
                acc = cvt.tile([128, SCW], f32, tag="cacc")
                nc.vector.tensor_scalar(
                    out=acc, in0=xpre[:, 0:SCW],
                    scalar1=convw_sb[:, m, 0:1], scalar2=convb_sb[:, m, 0:1],
                    op0=OP.mult, op1=OP.add)
                for k in range(1, D_CONV):
                    nc.vector.scalar_tensor_tensor(
                        out=acc, in0=xpre[:, k:SCW + k],
                        scalar=convw_sb[:, m, k:k + 1], op0=OP.mult,
                        in1=acc, op1=OP.add)
                nc.scalar.activation(out=xcv_sb[:, m, :], in_=acc,
                                     func=AT.Sigmoid)
                nc.vector.tensor_tensor(out=xcv_sb[:, m, :],
                                        in0=xcv_sb[:, m, :], in1=acc,
                                        op=OP.mult)

            # ---- row GEMM (z|dt) ----
            z_sb = zp.tile([128, QPS, 1024], bf)
            dt_sb = dsp.tile([128, QPS, NH], f32, tag="dt")
            s_sb = dsp.tile([128, QPS, NH], f32, tag="s")
            for mt in range(QPS):
                for (n0, nw) in ((0, 512), (512, 512), (1024, NH)):
                    ps = gps.tile([128, nw], f32, tag="gps")
                    for k in range(8):
                        nc.tensor.matmul(
                            ps, lhsT=xt_sb[:, k, mt * 128:(mt + 1) * 128],
                            rhs=wrow_sb[:, k, n0:n0 + nw],
                            start=(k == 0), stop=(k == 7))
                    if n0 < 1024:
                        nc.scalar.activation(
                            out=z_sb[:, mt, n0:n0 + nw], in_=ps,
                            func=AT.Sigmoid)
                        nc.vector.tensor_tensor(
                            out=z_sb[:, mt, n0:n0 + nw],
                            in0=z_sb[:, mt, n0:n0 + nw], in1=ps, op=OP.mult)
                    else:
                        tdt = ck2.tile([128, NH], f32, tag="tdt")
                        nc.vector.tensor_tensor(
                            out=tdt, in0=ps, in1=dtb_sb, op=OP.add)
                        nc.scalar.activation(out=tdt, in_=tdt,
                                             func=AT.Sigmoid, scale=-1.0)
                        nc.scalar.activation(out=tdt, in_=tdt, func=AT.Ln)
                        nc.vector.tensor_scalar_mul(
                            dt_sb[:, mt, :], tdt, -1.0)
                        nc.vector.tensor_tensor(
                            out=s_sb[:, mt, :], in0=dt_sb[:, mt, :], in1=abc_sb,
                            op=OP.mult)

            # ---- scan over the 4 chunks ----
            for q in range(QPS):
                c = sc * QPS + q
                qsl = slice(q * Q, (q + 1) * Q)

                # ell (in-chunk inclusive cumsum of s), per-head decay scalars
                ell_ps = sps.tile([128, NH], f32, tag="sp")
                nc.tensor.matmul(ell_ps, lhsT=tril_sb, rhs=s_sb[:, q, :],
                                 start=True, stop=True)
                ell = ck2.tile([128, NH], f32, tag="ell")
                nc.vector.tensor_copy(out=ell, in_=ell_ps)
                lamc_ps = sps.tile([1, NH], f32, tag="sp")
                nc.tensor.matmul(lamc_ps, lhsT=onesc_sb, rhs=s_sb[:, q, :],
                                 start=True, stop=True)
                lam1 = ck2.tile([1, NH], f32, tag="lam1")
                nc.vector.tensor_copy(out=lam1, in_=lamc_ps)
                lam_ps = sps.tile([128, NH], f32, tag="sp")
                nc.tensor.matmul(lam_ps, lhsT=ones1_sb, rhs=lam1,
                                 start=True, stop=True)
                g_b = ck2.tile([128, NH], f32, tag="gb")
                nc.scalar.activation(out=g_b, in_=lam_ps, func=AT.Exp)
                ew = ck2.tile([128, NH], f32, tag="ew")
                nc.vector.tensor_tensor(out=ew, in0=lam_ps, in1=ell, op=OP.subtract)
                nc.scalar.activation(out=ew, in_=ew, func=AT.Exp)
                erow = ck2.tile([128, NH], f32, tag="erow")
                nc.scalar.activation(out=erow, in_=ell, func=AT.Exp)

                # G^T masked (shared across heads)
                g_ps = sps.tile([128, 128], f32, tag="sp")
                nc.tensor.matmul(g_ps, lhsT=xcv_sb[:, 8, qsl],
                                 rhs=xcv_sb[:, 9, qsl], start=True, stop=True)
                gm = ck2.tile([128, 128], f32, tag="gm")
                nc.vector.tensor_tensor(out=gm, in0=g_ps, in1=tril_sb, op=OP.mult)

                # B rows (for dH)
                br_ps = sps.tile([128, 128], bf, tag="sp")
                nc.tensor.transpose(br_ps, xcv_sb[:, 8, qsl], identb_sb)
                brow = ck2.tile([128, 128], bf, tag="brow")
                nc.vector.tensor_copy(out=brow, in_=br_ps)

                # x transposes -> U rows, x rows
                xrow = ck2.tile([128, NH, HD], bf, tag="xrow")
                u_sb = ck2.tile([128, NH, HD], bf, tag="u")
                for j in range(8):
                    xT_ps = sps.tile([128, 128], bf, tag="sp")
                    nc.tensor.transpose(xT_ps, xcv_sb[:, j, qsl], identb_sb)
                    nc.vector.tensor_copy(
                        out=xrow[:, 2 * j:2 * j + 2, :],
                        in_=xT_ps.rearrange("p (h d) -> p h d", h=2))
                    nc.vector.tensor_tensor(
                        out=u_sb[:, 2 * j:2 * j + 2, :],
                        in0=xT_ps.rearrange("p (h d) -> p h d", h=2),
                        in1=dt_sb[:, q, 2 * j:2 * j + 2, None].to_broadcast(
                            [128, 2, HD]),
                        op=OP.mult)

                # decay matrices Mt[s',t] = exp(min(ell_t - ell_s', 0)) * Gm
                ellT_ps = sps.tile([NH, 128], f32, tag="sp")
                nc.tensor.transpose(ellT_ps, ell, identf_sb)
                ellT = ck2.tile([NH, 128], f32, tag="ellT")
                nc.vector.tensor_copy(out=ellT, in_=ellT_ps)
                ellscr = drp.tile([NH, 128], f32, tag="ellscr")
                nc.sync.dma_start(out=ellscr, in_=ellT)
                f_sb = ck1.tile([128, NH, 128], f32)
                nc.sync.dma_start(
                    out=f_sb,
                    in_=ellscr[None, :, :].to_broadcast([128, NH, 128]))
                dmin = ck1.tile([128, NH, 128], f32)
                nc.vector.tensor_tensor(
                    out=dmin, in0=f_sb,
                    in1=ell[:, :, None].to_broadcast([128, NH, 128]),
                    op=OP.subtract)
                nc.vector.tensor_scalar(out=dmin, in0=dmin, scalar1=0.0,
                                        scalar2=None, op0=OP.min)
                nc.scalar.activation(out=dmin, in_=dmin, func=AT.Exp)
                mt_sb = ck1.tile([128, NH, 128], bf)
                nc.vector.tensor_tensor(
                    out=mt_sb, in0=dmin,
                    in1=gm[:, None, :].to_broadcast([128, NH, 128]), op=OP.mult)

                # H snapshot in bf16 for this chunk's cross term
                hb = ck2.tile([128, NH, HD], bf, tag="hb")
                nc.vector.tensor_copy(out=hb, in_=H)

                # per-head matmuls: y_local, cross P
                y_ps = yps.tile([128, NH, HD], f32, tag="y")
                for h in range(NH):
                    nc.tensor.matmul(y_ps[:, h, :], lhsT=mt_sb[:, h, :],
                                     rhs=u_sb[:, h, :], start=True, stop=True)
                p_ps = yps.tile([128, NH, HD], f32, tag="pdh")
                for h in range(NH):
                    nc.tensor.matmul(p_ps[:, h, :], lhsT=xcv_sb[:, 9, qsl],
                                     rhs=hb[:, h, :], start=True, stop=True)

                # y = D*x + y_local + e_row*P
                ysb = ck1.tile([128, NH, HD], f32, tag="ysb")
                nc.vector.tensor_tensor(
                    out=ysb, in0=xrow,
                    in1=dbc_sb[:, :, None].to_broadcast([128, NH, HD]), op=OP.mult)
                nc.vector.tensor_tensor(out=ysb, in0=ysb, in1=y_ps, op=OP.add)
                ycr = ck1.tile([128, NH, HD], f32, tag="ycr")
                nc.vector.tensor_tensor(
                    out=ycr, in0=p_ps,
                    in1=erow[:, :, None].to_broadcast([128, NH, HD]), op=OP.mult)
                nc.vector.tensor_tensor(out=ysb, in0=ysb, in1=ycr, op=OP.add)

                # state update: H = g*H + B_row^T @ (ew*U)
                uw = ck2.tile([128, NH, HD], bf, tag="uw")
                nc.vector.tensor_tensor(
                    out=uw, in0=u_sb,
                    in1=ew[:, :, None].to_broadcast([128, NH, HD]), op=OP.mult)
                dh_ps = yps.tile([128, NH, HD], f32, tag="pdh")
                for h in range(NH):
                    nc.tensor.matmul(dh_ps[:, h, :], lhsT=brow,
                                     rhs=uw[:, h, :], start=True, stop=True)
                nc.vector.tensor_tensor(
                    out=H, in0=H,
                    in1=g_b[:, :, None].to_broadcast([128, NH, HD]), op=OP.mult)
                nc.vector.tensor_tensor(out=H, in0=H, in1=dh_ps, op=OP.add)

                # gating + ssq + out projection (partial)
                yg = ck2.tile([128, 1024], bf, tag="yg")
                nc.vector.tensor_tensor(
                    out=yg, in0=ysb.rearrange("p h d -> p (h d)"),
                    in1=z_sb[:, q, :], op=OP.mult)
                sq = ck1.tile([128, 1024], f32, tag="sq")
                nc.vector.tensor_tensor(out=sq, in0=yg, in1=yg, op=OP.mult)
                ssqt = ck2.tile([128, 1], f32, tag="ssqt")
                nc.vector.tensor_reduce(out=ssqt, in_=sq, axis=AX.X, op=OP.add)
                nc.sync.dma_start(out=ssqo[c * Q:(c + 1) * Q, :], in_=ssqt)

                ygT = ck2.tile([128, 8, 128], bf, tag="ygT")
                for j in range(8):
                    yT_ps = sps.tile([128, 128], bf, tag="sp")
                    nc.tensor.transpose(
                        yT_ps, yg[:, j * 128:(j + 1) * 128], identb_sb)
                    nc.vector.tensor_copy(out=ygT[:, j, :], in_=yT_ps)

                osb = ck2.tile([128, 1024], bf, tag="osb")
                for o in range(2):
                    ops_ = gps.tile([128, 512], f32, tag="gps")
                    for k in range(8):
                        nc.tensor.matmul(
                            ops_, lhsT=ygT[:, k, :],
                            rhs=wout_sb[:, k, o * 512:(o + 1) * 512],
                            start=(k == 0), stop=(k == 7))
                    nc.vector.tensor_copy(out=osb[:, o * 512:(o + 1) * 512],
                                          in_=ops_)
                nc.sync.dma_start(out=out[c * Q:(c + 1) * Q, :], in_=osb)

    return nc


# ---------------------------------------------------------------------------
# host packing
# ---------------------------------------------------------------------------

def _pack_inputs(embed_data, W_in, conv_w, conv_b, dt_bias, A_log, D, norm_w,
                 W_out, l_total=L, nb=B):
    import ml_dtypes
    bfd = ml_dtypes.bfloat16
    A_full = (-np.exp(A_log.astype(np.float32))).astype(np.float32)

    tril = np.triu(np.ones((128, 128), np.float32))
    identf = np.eye(128, dtype=np.float32)
    identb = identf.astype(bfd)
    ones1 = np.ones((1, 128), np.float32)

    in_maps = []
    for c in range(2 * nb):
        bi, hh = c // 2, c % 2
        hsl = slice(hh * NH, (hh + 1) * NH)
        csl = slice(hh * 1024, (hh + 1) * 1024)
        xtv = np.ascontiguousarray(
            embed_data[bi, :l_total].T).astype(bfd)                   # [1024, L]
        wrow = np.ascontiguousarray(np.concatenate(
            [W_in[csl], W_in[4352 + hh * NH:4352 + (hh + 1) * NH]], 0).T
        ).astype(bfd)                                                  # [1024,1040]
        wcol = np.ascontiguousarray(np.concatenate(
            [W_in[2048 + hh * 1024:2048 + (hh + 1) * 1024],
             W_in[4096:4224], W_in[4224:4352]], 0).T).astype(bfd)      # [1024,1280]
        woutv = np.ascontiguousarray(
            (W_out[:, csl] * norm_w[csl][None, :]).T).astype(bfd)      # [1024,1024]
        cw = np.ascontiguousarray(np.concatenate(
            [conv_w[hh * 1024:(hh + 1) * 1024], conv_w[2048:2304]], 0)
        ).astype(np.float32)
        cb = np.ascontiguousarray(np.concatenate(
            [conv_b[hh * 1024:(hh + 1) * 1024], conv_b[2048:2304]], 0)
        ).astype(np.float32)[:, None]
        in_maps.append({
            "xt": xtv, "wrow": wrow, "wcol": wcol, "wout": woutv,
            "convw": cw, "convb": cb,
            "dtb": np.ascontiguousarray(
                np.broadcast_to(dt_bias[hsl], (128, NH))).astype(np.float32),
            "abc": np.ascontiguousarray(
                np.broadcast_to(A_full[hsl], (128, NH))).astype(np.float32),
            "dbc": np.ascontiguousarray(
                np.broadcast_to(D[hsl], (128, NH))).astype(np.float32),
            "tril": tril, "identb": identb, "identf": identf, "ones1": ones1,
            "onesc": np.ones((128, 1), np.float32),
        })
    return in_maps


# ---------------------------------------------------------------------------
# entry point
# ---------------------------------------------------------------------------

def _get_runner():
    """Build the sharded-jit runner once (nc trace + jit closure)."""
    if "runner" in _CACHE:
        return _CACHE["runner"]
    import time as _t
    import sys as _sys
    _t0 = _t.perf_counter()
    import jax
    import jax.numpy as jnp
    from jax.sharding import Mesh, NamedSharding, PartitionSpec
    from jax.experimental.shard_map import shard_map
    import concourse.bass2jax as b2j
    import concourse.mybir as mybir
    _t1 = _t.perf_counter()

    nc = _build_nc(L)
    _t2 = _t.perf_counter()

    b2j.install_neuronx_cc_hook()
    partition_name = (nc.partition_id_tensor.name
                      if nc.partition_id_tensor else None)
    in_names, out_names, out_avals, in_avals = [], [], [], []
    for alloc in nc.m.functions[0].allocations:
        if not isinstance(alloc, mybir.MemoryLocationSet):
            continue
        name = alloc.memorylocations[0].name
        if alloc.kind == "ExternalInput":
            if name != partition_name:
                in_names.append(name)
                in_avals.append(jax.core.ShapedArray(
                    tuple(alloc.tensor_shape), mybir.dt.np(alloc.dtype)))
        elif alloc.kind == "ExternalOutput":
            out_names.append(name)
            out_avals.append(jax.core.ShapedArray(
                tuple(alloc.tensor_shape), mybir.dt.np(alloc.dtype)))
    n_params, n_outs = len(in_names), len(out_avals)
    all_in = in_names + out_names + ([partition_name] if partition_name else [])

    def _body(*args):
        ops = list(args)
        if partition_name:
            ops.append(b2j.partition_id_tensor())
        outs = b2j._bass_exec_p.bind(
            *ops, out_avals=tuple(out_avals), in_names=tuple(all_in),
            out_names=tuple(out_names), lowering_input_output_aliases=(),
            sim_require_finite=True, sim_require_nnan=True, nc=nc)
        return tuple(outs)

    devices = jax.devices()[:8]
    mesh = Mesh(np.asarray(devices), ("core",))
    spec = NamedSharding(mesh, PartitionSpec("core"))
    in_specs = (PartitionSpec("core"),) * (n_params + n_outs)
    out_specs = (PartitionSpec("core"),) * n_outs
    donate = tuple(range(n_params, n_params + n_outs))
    sharded = jax.jit(
        shard_map(_body, mesh=mesh, in_specs=in_specs, out_specs=out_specs,
                  check_rep=False),
        donate_argnums=donate, keep_unused=True)
    zshapes = [(8 * a.shape[0], *a.shape[1:]) for a in out_avals]
    zdtypes = [a.dtype for a in out_avals]
    make_zeros = jax.jit(
        lambda: tuple(jnp.zeros(s, d) for s, d in zip(zshapes, zdtypes)),
        out_shardings=tuple(spec for _ in out_avals))

    runner = {
        "jax": jax, "sharded": sharded, "make_zeros": make_zeros,
        "spec": spec, "in_names": in_names, "out_names": out_names,
        "out_avals": out_avals, "in_avals": in_avals,
    }
    _CACHE["runner"] = runner
    return runner


def _get_prep_jits():
    """Device-side prep (dedup uploads via all-gather) and combine jits.
    Export-cached to disk so their compiled NEFFs are reused regardless of
    the directory kernel.py runs from (jax module hashes embed file paths)."""
    if "prep" in _CACHE:
        return _CACHE["prep"]
    import jax
    import jax.numpy as jnp
    from jax import export as jexport
    from jax.sharding import Mesh, NamedSharding, PartitionSpec as P
    from jax.experimental.shard_map import shard_map

    mesh = Mesh(np.asarray(jax.devices()[:8]), ("core",))
    spec = NamedSharding(mesh, P("core"))
    path = _export_cache_path() + "_pc"
    if os.path.exists(path + "_prep.bin") and os.path.exists(path + "_comb.bin"):
        prep_j = jax.jit(jexport.deserialize(
            open(path + "_prep.bin", "rb").read()).call)
        comb_j = jax.jit(jexport.deserialize(
            open(path + "_comb.bin", "rb").read()).call)
        _CACHE["prep"] = (prep_j, comb_j, spec)
        return _CACHE["prep"]

    PAIRS = [[0, 1], [2, 3], [4, 5], [6, 7]]
    QUADS = [[0, 2, 4, 6], [1, 3, 5, 7]]

    def prep(eh, wrq, wcq, woq):
        # eh [2048, 1024]: this core's L-half of its batch's bf16 embedding
        xt = jax.lax.all_gather(eh, "core", axis=0, axis_index_groups=PAIRS)
        xt = xt.reshape(4096, 1024).T
        # w*q [256, n]: this core's quarter of its head-half's weights
        wr = jax.lax.all_gather(wrq, "core", axis=0,
                                axis_index_groups=QUADS).reshape(1024, -1)
        wc = jax.lax.all_gather(wcq, "core", axis=0,
                                axis_index_groups=QUADS).reshape(1024, -1)
        wo = jax.lax.all_gather(woq, "core", axis=0,
                                axis_index_groups=QUADS).reshape(1024, -1)
        # donated zero output buffers for the bass call, made device-side
        zo = jnp.zeros((L, 1024), jnp.bfloat16)
        zs = jnp.zeros((L, 1), jnp.float32)
        return xt, wr, wc, wo, zo, zs

    prep_j = jax.jit(shard_map(
        prep, mesh=mesh, in_specs=(P("core"),) * 4, out_specs=(P("core"),) * 6))

    def comb(o, s):
        og = jax.lax.all_gather(o.astype(jnp.float32), "core",
                                axis_index_groups=PAIRS)
        sg = jax.lax.all_gather(s, "core", axis_index_groups=PAIRS)
        rr = jax.lax.rsqrt(sg.sum(0) / 2048.0 + 1e-5)
        res = (og.sum(0) * rr).astype(jnp.bfloat16)
        idx = jax.lax.axis_index("core") % 2
        return jax.lax.dynamic_slice_in_dim(res, idx * 2048, 2048, 0)

    comb_j = jax.jit(shard_map(
        comb, mesh=mesh, in_specs=(P("core"), P("core")), out_specs=P("core")))

    try:
        bfd = jnp.bfloat16
        prep_avals = (jax.ShapeDtypeStruct((8 * 2048, DM), bfd),
                      jax.ShapeDtypeStruct((8 * 256, 1040), bfd),
                      jax.ShapeDtypeStruct((8 * 256, 1280), bfd),
                      jax.ShapeDtypeStruct((8 * 256, 1024), bfd))
        comb_avals = (jax.ShapeDtypeStruct((8 * L, 1024), bfd),
                      jax.ShapeDtypeStruct((8 * L, 1), np.float32))
        with open(path + "_prep.bin", "wb") as f:
            f.write(jexport.export(prep_j)(*prep_avals).serialize())
        with open(path + "_comb.bin", "wb") as f:
            f.write(jexport.export(comb_j)(*comb_avals).serialize())
        # use the deserialized variants so their (path-independent) module
        # hashes are the ones warmed in the persistent compile cache
        prep_j = jax.jit(jexport.deserialize(
            open(path + "_prep.bin", "rb").read()).call)
        comb_j = jax.jit(jexport.deserialize(
            open(path + "_comb.bin", "rb").read()).call)
    except Exception:
        pass

    _CACHE["prep"] = (prep_j, comb_j, spec)
    return _CACHE["prep"]


def _export_cache_path():
    import hashlib
    import inspect
    src = inspect.getsource(_build_nc) + inspect.getsource(_get_prep_jits)
    h = hashlib.sha256(src.encode()).hexdigest()[:12]
    d = os.path.expanduser("~/.cache/mamba_trn")
    os.makedirs(d, exist_ok=True)
    return os.path.join(d, f"exp_{h}")


def _get_exec():
    """Jitted bass computation: from the export cache if present, else built
    fresh (and then cached). Returns (fn, in_names, out_names, out_gshapes)."""
    if "exec" in _CACHE:
        return _CACHE["exec"]
    import json
    import jax
    from jax import export as jexport
    import concourse.bass2jax as b2j
    b2j.BassEffect.__eq__ = lambda self, other: type(self) is type(other)
    b2j.BassEffect.__hash__ = lambda self: hash(type(self))
    b2j.install_neuronx_cc_hook()

    path = _export_cache_path()
    if os.path.exists(path + ".bin") and os.path.exists(path + ".json"):
        meta = json.load(open(path + ".json"))
        exp = jexport.deserialize(open(path + ".bin", "rb").read())
        fn = jax.jit(exp.call)
        res = (fn, meta["in_names"], meta["out_names"],
               [tuple(s) for s in meta["out_gshapes"]],
               [np.dtype(d) for d in meta["out_dtypes"]])
    else:
        r = _get_runner()
        avals = [jax.ShapeDtypeStruct((8 * a.shape[0], *a.shape[1:]), a.dtype)
                 for a in r["in_avals"]]
        zavals = [jax.ShapeDtypeStruct((8 * a.shape[0], *a.shape[1:]), a.dtype)
                  for a in r["out_avals"]]
        gshapes = [tuple(z.shape) for z in zavals]
        gdtypes = [str(np.dtype(z.dtype)) for z in zavals]
        fn = r["sharded"]
        try:
            exp = jexport.export(r["sharded"], disabled_checks=[
                jexport.DisabledSafetyCheck.custom_call("bass_exec")])(
                *avals, *zavals)
            with open(path + ".bin", "wb") as f:
                f.write(exp.serialize())
            with open(path + ".json", "w") as f:
                json.dump({"in_names": r["in_names"],
                           "out_names": r["out_names"],
                           "out_gshapes": gshapes,
                           "out_dtypes": gdtypes}, f)
            fn = jax.jit(jexport.deserialize(
                open(path + ".bin", "rb").read()).call)
        except Exception:
            pass
        res = (fn, r["in_names"], r["out_names"], gshapes,
               [np.dtype(z.dtype) for z in zavals])
    _CACHE["exec"] = res
    return res


def _run_on_trn(embed_data, W_in, conv_w, conv_b, dt_bias, A_log, D, norm_w,
                W_out):
    import time as _t
    import sys as _sys
    import ml_dtypes
    import jax
    bfd = ml_dtypes.bfloat16

    _t0 = _t.perf_counter()
    prep_j, comb_j, spec = _get_prep_jits()

    # ---- pack + async upload (embed conversion runs in a worker thread) ----
    from concurrent.futures import ThreadPoolExecutor
    pool = ThreadPoolExecutor(1)
    fut_eh = pool.submit(
        lambda: jax.device_put(
            embed_data.astype(bfd).reshape(8 * 2048, DM), spec))

    A_full = (-np.exp(A_log.astype(np.float32))).astype(np.float32)
    wrow_h, wcol_h, wout_h = [], [], []
    for hh in range(2):
        csl = slice(hh * 1024, (hh + 1) * 1024)
        wrow_h.append(np.concatenate(
            [W_in[csl], W_in[4352 + hh * NH:4352 + (hh + 1) * NH]], 0)
            .T.astype(bfd))                                # [1024, 1040]
        wcol_h.append(np.concatenate(
            [W_in[2048 + hh * 1024:2048 + (hh + 1) * 1024],
             W_in[4096:4224], W_in[4224:4352]], 0).T.astype(bfd))
        wout_h.append((W_out[:, csl] * norm_w[csl][None, :]).T.astype(bfd))

    def quarters(ws):
        return np.concatenate(
            [ws[c % 2][256 * (c // 2):256 * (c // 2 + 1)] for c in range(8)], 0)

    up = {
        "wrq": jax.device_put(quarters(wrow_h), spec),
        "wcq": jax.device_put(quarters(wcol_h), spec),
        "woq": jax.device_put(quarters(wout_h), spec),
    }

    tril = np.triu(np.ones((128, 128), np.float32))
    identf = np.eye(128, dtype=np.float32)
    smalls = {}
    for hh in range(2):
        hsl = slice(hh * NH, (hh + 1) * NH)
        smalls[hh] = {
            "convw": np.concatenate(
                [conv_w[hh * 1024:(hh + 1) * 1024], conv_w[2048:2304]],
                0).astype(np.float32),
            "convb": np.concatenate(
                [conv_b[hh * 1024:(hh + 1) * 1024], conv_b[2048:2304]],
                0).astype(np.float32)[:, None],
            "dtb": np.ascontiguousarray(np.broadcast_to(
                dt_bias[hsl], (128, NH))).astype(np.float32),
            "abc": np.ascontiguousarray(np.broadcast_to(
                A_full[hsl], (128, NH))).astype(np.float32),
            "dbc": np.ascontiguousarray(np.broadcast_to(
                D[hsl], (128, NH))).astype(np.float32),
        }
    shared = {"tril": tril, "identb": identf.astype(bfd), "identf": identf,
              "ones1": np.ones((1, 128), np.float32),
              "onesc": np.ones((128, 1), np.float32)}
    small_up = {}
    for n in ("convw", "convb", "dtb", "abc", "dbc"):
        small_up[n] = jax.device_put(
            np.concatenate([smalls[c % 2][n] for c in range(8)], 0), spec)
    for n, v in shared.items():
        small_up[n] = jax.device_put(
            np.concatenate([v] * 8, 0), spec)
    _t1 = _t.perf_counter()

    # ---- bass computation (export cache avoids re-tracing the program) ----
    fn, in_names, out_names, out_gshapes, out_gdtypes = _get_exec()
    _t2 = _t.perf_counter()

    xt_g, wr_g, wc_g, wo_g, zo, zs = prep_j(
        fut_eh.result(), up["wrq"], up["wcq"], up["woq"])
    big = {"xt": xt_g, "wrow": wr_g, "wcol": wc_g, "wout": wo_g}
    concat_in = [big[n] if n in big else small_up[n] for n in in_names]
    zeros = {"out": zo, "ssq": zs}
    outs = fn(*concat_in, *[zeros[n] for n in out_names])
    by_name = dict(zip(out_names, outs))
    final = comb_j(by_name["out"], by_name["ssq"])
    resv = np.asarray(final)

    return np.ascontiguousarray(
        resv.reshape(B, L, DM).astype(np.float32))


def _numpy_fallback(embed_data, W_in, conv_w, conv_b, dt_bias, A_log, D,
                    norm_w, W_out):
    """Vectorized chunked-SSD fallback (float32, CPU)."""
    b, l, _ = embed_data.shape
    d_inner, nheads, headdim = 2048, 32, 64
    d_state, nch = 128, l // Q
    zxbcdt = embed_data.reshape(b * l, DM) @ W_in.T
    zxbcdt = zxbcdt.reshape(b, l, -1)
    z = zxbcdt[..., :d_inner]
    xBC = zxbcdt[..., d_inner:d_inner + 2304]
    dtr = zxbcdt[..., d_inner + 2304:]
    xpad = np.concatenate([np.zeros((b, 3, 2304), np.float32), xBC], 1)
    xc = conv_b + sum(conv_w[:, k] * xpad[:, k:k + l] for k in range(D_CONV))
    xBC = xc / (1.0 + np.exp(-xc))
    xs = xBC[..., :d_inner].reshape(b, l, nheads, headdim)
    Bm = xBC[..., d_inner:d_inner + d_state]
    Cm = xBC[..., d_inner + d_state:]
    dt = np.where(dtr + dt_bias > 20.0, dtr + dt_bias,
                  np.log1p(np.exp(np.minimum(dtr + dt_bias, 20.0))))
    s = dt * (-np.exp(A_log))
    trilm = np.tril(np.ones((Q, Q), np.float32))
    y = np.empty((b, nch, Q, nheads, headdim), np.float32)
    for bi in range(b):
        ell = np.cumsum(s[bi].reshape(nch, Q, nheads), axis=1)
        lam = ell[:, -1, :]
        U = (dt[bi, :, :, None] * xs[bi]).reshape(nch, Q, nheads, headdim)
        B_c = Bm[bi].reshape(nch, Q, d_state)
        C_c = Cm[bi].reshape(nch, Q, d_state)
        G = np.einsum('ctn,csn->cts', C_c, B_c)
        Lm = np.exp(np.minimum(ell[:, :, None, :] - ell[:, None, :, :], 0.0))
        M = G[..., None] * Lm * trilm[None, :, :, None]
        yb = np.einsum('ctsh,cshp->cthp', M, U)
        dH = np.einsum('ctn,cth,cthp->chnp', B_c,
                       np.exp(lam[:, None, :] - ell), U)
        Hs = np.zeros((nheads, d_state, headdim), np.float32)
        expell = np.exp(ell)
        for c in range(nch):
            yb[c] += np.einsum('tn,hnp,th->thp', C_c[c], Hs, expell[c])
            Hs = np.exp(lam[c])[:, None, None] * Hs + dH[c]
        y[bi] = yb + D[None, None, :, None] * xs[bi].reshape(
            nch, Q, nheads, headdim)
    y = y.reshape(b, l, d_inner)
    y = y * (z / (1.0 + np.exp(-z)))
    ms = np.mean(np.square(y), axis=-1, keepdims=True)
    y = y / np.sqrt(ms + 1e-5) * norm_w
    return (y.reshape(b * l, d_inner) @ W_out.T).reshape(b, l, DM)


def kernel(embed_data, W_in, conv_w, conv_b, dt_bias, A_log, D, norm_w, W_out):
    args = [np.asarray(a, dtype=np.float32) for a in (
        embed_data, W_in, conv_w, conv_b, dt_bias, A_log, D, norm_w, W_out)]
    if os.environ.get("MAMBA_FORCE_NUMPY"):
        return _numpy_fallback(*args)
    try:
        return _run_on_trn(*args)
    except Exception:
        import traceback
        traceback.print_exc()
        return _numpy_fallback(*args)
